# revision 20
# baseline (speedup 1.0000x reference)
"""Causal self-attention (B=2, T=2048, C=1024, 16 heads) on 8 trn2 NeuronCores.

Sharding: tensor-parallel, core c = b*4+g handles batch b (2) x head-group g
(4 heads = 256 channels). Each core computes q/k/v projections for its
channels, causal attention for its 4 heads, and the slice of the output
projection contracting its channels. Host sums the 4 partial outputs per
batch. No cross-core communication on device.

Single-core schedule is built around keeping the PE array continuously fed
(the TRN2 PE DVFS ramp only reaches full clock after ~3us of uninterrupted
work): attention score/AV matmuls for a head pair are pipelined with lag-1
against the ACT-engine exps, and projection / output-projection matmuls are
dribbled between attention steps as filler. All matmul operands are fp16.
"""

import sys

if "/opt/trn_rl_repo" not in sys.path:
    sys.path.insert(0, "/opt/trn_rl_repo")

import numpy as np

import concourse.bass as bass
import concourse.mybir as mybir
from concourse.bass_utils import run_bass_kernel_spmd
from concourse.tile import TileContext
import concourse.tile_utils as _tile_utils

_tile_utils.max_sbuf_usage = 208 * 1024
from concourse.vector_clock import ScopedClock

# ---------------------------------------------------------------------------
# Walrus on this image rejects >4 sem waits on a single instruction; the stock
# TileContext tail-drain attaches one wait per active logical processor.
# Split them into standalone wait_ge instructions instead.
def _patched_drain_and_barrier(self, tick_clock, wait_clock):
    probe = mybir.InstNoOp(name="wait_probe", ins=[], outs=[])
    probe.engine = mybir.EngineType.SP
    wait_clock.add_sem_waits(probe, ScopedClock({None: tick_clock.global_clock}))
    waits = (
        list(probe.sync_info.on_wait)
        if probe.sync_info and probe.sync_info.on_wait
        else []
    )
    assert self.sems is not None
    sem_by_num = {s.num: s for s in self.sems.allocated().values()}
    for w in waits:
        assert w.wait_mode == "sem-ge-imm", w
        self.nc.sync.wait_ge(sem_by_num[w.id], w.wait_value)
    self.nc.sync.drain()
    self.nc.all_engine_barrier()
    popped = self.nc._tile_sem_poison_stack.pop()
    assert popped is self._sem_poison
    self.nc.clear_and_free_semaphores(list(self.sems.allocated().values()))
    self.nc.all_engine_barrier()


TileContext._drain_and_barrier = _patched_drain_and_barrier

# The same walrus limit applies to regular instructions (matmul/LDWEIGHTS
# rejects even 2 waits). Split multi-wait instructions: excess waits move to
# single-wait NoOps committed just before on the same engine.
_orig_commit = TileContext._commit_instruction


def _split_commit(self, inst, lazy_reg_writes=True):
    si = inst.sync_info
    if (
        si is not None
        and si.on_wait
        and len(si.on_wait) > 1
        and inst.engine != mybir.EngineType.Unassigned
    ):
        waits = list(si.on_wait)
        for w in waits[:-1]:
            nop = mybir.InstNoOp(
                name=self.nc.get_next_instruction_name(),
                ins=[],
                outs=[],
                engine=inst.engine,
                sync_info=mybir.SyncInfo(on_wait=[w], on_update=[]),
                bass_nofuse=True,
            )
            _orig_commit(self, nop, lazy_reg_writes=False)
        inst.sync_info = mybir.SyncInfo(
            on_wait=[waits[-1]], on_update=list(si.on_update or [])
        )
    _orig_commit(self, inst, lazy_reg_writes)


TileContext._commit_instruction = _split_commit
# ---------------------------------------------------------------------------

N_CORES = 8
B, T, C = 2, 2048, 1024
H = 16
DH = C // H                       # 64
HPC = H // 4                      # 4 heads per core
CS = HPC * DH                     # 256 channels per core
SCALE = 1.0 / np.sqrt(np.float32(C))  # note: sqrt(n_embd), per reference

P = 128                           # partitions
TB = T // P                       # 16 t-blocks of 128
QC = T // 512                     # 4 q-chunks of 512
KO = C // P                       # 8 contraction subtiles for projections

F32 = mybir.dt.float32
F16 = mybir.dt.float16

TRACE = False        # test.py flips this to profile
TRACE_KWARGS = {}
LAST_RESULT = None   # BassKernelResults of the most recent run

_NC_CACHE = None


def _build_nc():
    nc = bass.Bass()

    xT_d = nc.dram_tensor("xT", [C, T], F16, kind="ExternalInput")
    wqT_d = nc.dram_tensor("wqT", [C, CS], F16, kind="ExternalInput")
    wkT_d = nc.dram_tensor("wkT", [C, CS], F16, kind="ExternalInput")
    wvT_d = nc.dram_tensor("wvT", [C, CS], F16, kind="ExternalInput")
    peT_d = nc.dram_tensor("peT", [CS, T], F16, kind="ExternalInput")
    pen_d = nc.dram_tensor("pen", [T, CS], F16, kind="ExternalInput")
    woT_d = nc.dram_tensor("woT", [CS, C], F16, kind="ExternalInput")
    mask_d = nc.dram_tensor("maskT", [P, P], F16, kind="ExternalInput")
    out_d = nc.dram_tensor("out", [T, C], F16, kind="ExternalOutput")

    with TileContext(nc) as tc:
        with (
            nc.allow_low_precision(reason="fp16 matmul pipeline by design"),
            tc.tile_pool(name="const", bufs=1) as const,
            tc.tile_pool(name="ptp", bufs=6) as ptpool,
            tc.tile_pool(name="yup", bufs=6) as yupool,
            tc.tile_pool(name="recp", bufs=2) as recpool,
            tc.tile_pool(name="oddp", bufs=2) as oddpool,
            tc.tile_pool(name="outp", bufs=3) as outpool,
            tc.tile_pool(name="stps", bufs=3, space="PSUM") as stps,
            tc.tile_pool(name="yps", bufs=2, space="PSUM") as yps,
            tc.tile_pool(name="fillps", bufs=3, space="PSUM") as fillps,
            tc.tile_pool(name="scratch", bufs=2, space="DRAM") as drampool,
        ):
            # ---- persistent tiles -------------------------------------------------
            wq_t = const.tile([P, KO, CS], F16, tag="wq")
            wk_t = const.tile([P, KO, CS], F16, tag="wk")
            wv_t = const.tile([P, KO, CS], F16, tag="wv")
            wo_t = const.tile([P, 2, C], F16, tag="wo")
            peT_t = const.tile([P, 2, T], F16, tag="peT")
            pen_t = const.tile([P, TB, CS], F16, tag="pen")
            mask_t = const.tile([P, P], F16, tag="mask")
            qT_t = const.tile([P, 2, T], F16, tag="qT")
            kT_t = const.tile([P, 2, T], F16, tag="kT")
            # per-head v with a trailing ones column ([v|1]) for row sums
            v_t = const.tile([P, TB, HPC, DH + 1], F16, tag="v")
            yTp_t = const.tile([P, 2, T], F16, tag="yTp")
            ones_t = const.tile([P, DH], F16, tag="ones")

            x_ts = [const.tile([P, KO, 512], F16, tag=f"x{n}", name=f"x_{n}")
                    for n in range(QC)]

            xT_r = xT_d.rearrange("(o p) t -> p o t", p=P)

            # DMA order = need order; first x chunk + wq halves split so the
            # first projection matmuls can start ASAP; x0 goes through the ACT
            # engine's DGE so its config isn't serialized behind the weights'
            # on the SP sequencer.
            wq_r = wqT_d.rearrange("(o p) m -> p o m", p=P)
            nc.scalar.dma_start(out=x_ts[0][:, 0:4, :], in_=xT_r[:, 0:4, 0:512])
            nc.sync.dma_start(out=wq_t[:, :, 0:P], in_=wq_r[:, :, 0:P])
            nc.gpsimd.dma_start(out=peT_t[:], in_=peT_d.rearrange("(o p) m -> p o m", p=P))
            nc.scalar.dma_start(out=x_ts[0][:, 4:8, :], in_=xT_r[:, 4:8, 0:512])
            nc.sync.dma_start(out=wq_t[:, :, P:CS], in_=wq_r[:, :, P:CS])
            nc.sync.dma_start(out=wk_t[:], in_=wkT_d.rearrange("(o p) m -> p o m", p=P))
            nc.gpsimd.dma_start(out=pen_t[:], in_=pen_d.rearrange("(o p) m -> p o m", p=P))
            nc.sync.dma_start(out=wv_t[:], in_=wvT_d.rearrange("(o p) m -> p o m", p=P))
            for n in range(1, QC):
                nc.sync.dma_start(out=x_ts[n][:], in_=xT_r[:, :, n * 512:(n + 1) * 512])
            nc.sync.dma_start(out=mask_t[:], in_=mask_d[:, :])
            nc.sync.dma_start(out=wo_t[:], in_=woT_d.rearrange("(o p) m -> p o m", p=P))

            nc.gpsimd.memset(ones_t[:], 1.0)
            nc.gpsimd.memset(v_t[:, :, :, DH:DH + 1], 1.0)
            # warm the combined ln/exp ACT table during the input-DMA wait so
            # the first real exp doesn't pay the 1.3us table load
            warm_t = const.tile([1, 2], F32, tag="warm")
            nc.scalar.activation(warm_t[0:1, 0:1], ones_t[0:1, 0:1],
                                 mybir.ActivationFunctionType.Exp)
            nc.scalar.activation(warm_t[0:1, 1:2], ones_t[0:1, 0:1],
                                 mybir.ActivationFunctionType.Ln)

            # ---- filler streams ---------------------------------------------------
            # Each filler step is a zero-arg callable emitting ONE instruction
            # (matmul / vector drain / dma). Attention steps consume these to
            # keep the PE queue backlogged with independent work.

            def proj_chunk_steps(n, qk=True, v_range=(0, 4)):
                """q/k/v projections for t-chunk n: list of step callables.

                qk=False skips the q/k units; v_range selects which v units
                to emit (v units can be deferred into the owning q-chunk's
                attention steps since AV needs v[tb] only at step tb).
                """
                steps = []
                ts = slice(n * 512, (n + 1) * 512)
                x_t = x_ts[n]

                qk_pairs = [(wq_t, qT_t), (wk_t, kT_t)] if qk else []
                for (w_t, dst) in qk_pairs:
                    for m in range(2):
                        box = {}

                        def mk_mm(ko, w_t=w_t, m=m, box=box):
                            def f():
                                if ko == 0:
                                    box["ps"] = fillps.tile(
                                        [P, 512], F32, tag="fill", name="psqk")
                                nc.tensor.matmul(
                                    box["ps"][:],
                                    lhsT=w_t[:, ko, m * P:(m + 1) * P],
                                    rhs=x_t[:, ko, :],
                                    start=(ko == 0),
                                    stop=(ko == KO - 1),
                                )
                            return f

                        for ko in range(KO):
                            steps.append(mk_mm(ko))

                        def mk_drain(dst=dst, m=m, ts=ts, box=box):
                            def f():
                                nc.vector.tensor_add(
                                    out=dst[:, m, ts], in0=box["ps"][:],
                                    in1=peT_t[:, m, ts],
                                )
                            return f

                        steps.append(mk_drain())

                for tb4 in range(*v_range):
                    tb = n * 4 + tb4
                    box = {}

                    def mk_mmv(ko, tb4=tb4, box=box):
                        def f():
                            if ko == 0:
                                box["ps"] = fillps.tile(
                                    [P, 512], F32, tag="fill", name="psv")
                            nc.tensor.matmul(
                                box["ps"][:, :CS],
                                lhsT=x_t[:, ko, tb4 * P:(tb4 + 1) * P],
                                rhs=wv_t[:, ko, :],
                                start=(ko == 0),
                                stop=(ko == KO - 1),
                            )
                        return f

                    for ko in range(KO):
                        steps.append(mk_mmv(ko))

                    def mk_drainv(tb=tb, box=box):
                        def f():
                            nc.vector.tensor_add(
                                out=v_t[:, tb, :, 0:DH],
                                in0=box["ps"][:, :CS].rearrange(
                                    "p (a b) -> p a b", a=HPC),
                                in1=pen_t[:, tb, :].rearrange(
                                    "p (a b) -> p a b", a=HPC),
                            )
                        return f

                    steps.append(mk_drainv())
                return steps

            def norm_steps(qc, rec_all):
                """Deferred normalization for q-chunk qc (bcast + muls).
                Odd heads are normalized into a staging tile then partition-
                shifted to rows 64-127 of yTp via SBUF->SBUF DMA."""
                qs = slice(qc * 512, (qc + 1) * 512)
                steps = []
                for hp in range(2):
                    box = {}

                    def mk_bc(hp=hp, box=box):
                        def f():
                            box["bcA"] = fillps.tile([P, 512], F32, tag="fill",
                                                     name="bcA")
                            box["bcB"] = fillps.tile([P, 512], F32, tag="fill",
                                                     name="bcB")
                            nc.tensor.matmul(
                                box["bcA"][0:DH, :], lhsT=ones_t[0:1, :],
                                rhs=rec_all[0:1, 2 * hp, :],
                                start=True, stop=True,
                            )
                            nc.tensor.matmul(
                                box["bcB"][0:DH, :], lhsT=ones_t[0:1, :],
                                rhs=rec_all[0:1, 2 * hp + 1, :],
                                start=True, stop=True,
                            )
                        return f

                    def mk_mul(hp=hp, box=box):
                        yuA, yuB = yu_tiles[qc][2 * hp], yu_tiles[qc][2 * hp + 1]

                        def f():
                            nc.vector.tensor_mul(
                                out=yTp_t[0:DH, hp, qs],
                                in0=yuA[0:DH, :], in1=box["bcA"][0:DH, :],
                            )
                            ytn = oddpool.tile([DH, 512], F16, tag="ytn",
                                               name=f"ytn_{qc}_{hp}")
                            nc.vector.tensor_mul(
                                out=ytn[:], in0=yuB[0:DH, :],
                                in1=box["bcB"][0:DH, :],
                            )
                            nc.sync.dma_start(
                                out=yTp_t[DH:P, hp, qs], in_=ytn[:])
                        return f

                    steps.append(mk_bc())
                    steps.append(mk_mul())
                return steps

            def outproj_steps(qc):
                """Output projection for q-chunk qc: 8 units of (2mm+drain)."""
                steps = []
                for tb4 in range(4):
                    tb = qc * 4 + tb4
                    tsl = slice(tb * P, (tb + 1) * P)
                    for oc in range(2):
                        box = {}
                        ocs = slice(oc * 512, (oc + 1) * 512)

                        def mk_mm(m, tsl=tsl, ocs=ocs, box=box):
                            def f():
                                if m == 0:
                                    box["ps"] = fillps.tile(
                                        [P, 512], F32, tag="fill", name="pso")
                                nc.tensor.matmul(
                                    box["ps"][:],
                                    lhsT=yTp_t[:, m, tsl],
                                    rhs=wo_t[:, m, ocs],
                                    start=(m == 0),
                                    stop=(m == 1),
                                )
                            return f

                        def mk_drain(tb=tb, oc=oc, tsl=tsl, ocs=ocs, box=box):
                            def f():
                                o_t = outpool.tile([P, 512], F16, tag="out",
                                                   name=f"o_{tb}_{oc}")
                                nc.vector.tensor_copy(out=o_t[:], in_=box["ps"][:])
                                nc.sync.dma_start(out=out_d[tsl, ocs], in_=o_t[:])
                            return f

                        steps.append(mk_mm(0))
                        steps.append(mk_mm(1))
                        steps.append(mk_drain())
                return steps


            def inline_norm_pair(qc, hp, yuA, yuB, yps_pair, reserve):
                """qc=3 normalization for pair hp, engine-local (no DMA
                bounce): reciprocal_approx_fast on the sums lane (partition
                64), fp16 copy, ones-bcast matmul at base partition 64.
                `reserve` = ready PE filler steps interleaved to cover the
                cross-engine latency."""
                qs = slice(qc * 512, (qc + 1) * 512)
                ri = iter(reserve)

                def take(k):
                    for _ in range(k):
                        f = next(ri, None)
                        if f is not None:
                            f()

                # 1/s = exp(-ln(s)) on the ACT engine: stays on the sums
                # lane (partition 64), no relayout, and the
                # natural_log_exp_and_others act table covers both functions
                rr, rf = {}, {}
                for tag, ysrc in (("A", yps_pair[0]), ("B", yps_pair[1])):
                    rr[tag] = recpool.tile([DH + 1, 512], F32, tag="rr",
                                           name=f"rr{tag}_{hp}")
                    nc.scalar.activation(
                        rr[tag][DH:DH + 1, :], ysrc[DH:DH + 1, :],
                        mybir.ActivationFunctionType.Ln,
                    )
                take(3)
                for tag in ("A", "B"):
                    rf[tag] = recpool.tile([DH + 1, 512], F16, tag="rf",
                                           name=f"rf{tag}_{hp}")
                    nc.scalar.activation(
                        rf[tag][DH:DH + 1, :], rr[tag][DH:DH + 1, :],
                        mybir.ActivationFunctionType.Exp, scale=-1.0,
                    )
                take(3)
                bcA = fillps.tile([P, 512], F32, tag="fill", name="bcA3")
                bcB = fillps.tile([P, 512], F32, tag="fill", name="bcB3")
                nc.tensor.matmul(bcA[0:DH, :], lhsT=ones_t[DH:DH + 1, :],
                                 rhs=rf["A"][DH:DH + 1, :],
                                 start=True, stop=True)
                nc.tensor.matmul(bcB[0:DH, :], lhsT=ones_t[DH:DH + 1, :],
                                 rhs=rf["B"][DH:DH + 1, :],
                                 start=True, stop=True)
                nc.vector.tensor_mul(
                    out=yTp_t[0:DH, hp, qs], in0=yuA[0:DH, :], in1=bcA[0:DH, :])
                ytn = oddpool.tile([DH, 512], F16, tag="ytn",
                                   name=f"ytn3_{hp}")
                nc.vector.tensor_mul(out=ytn[:], in0=yuB[0:DH, :],
                                     in1=bcB[0:DH, :])
                nc.sync.dma_start(out=yTp_t[DH:P, hp, qs], in_=ytn[:])
                take(9)

            yu_tiles = [[None] * HPC for _ in range(QC)]
            rec_alls = [None] * QC

            # ---- PE warm-up -------------------------------------------------------
            # The PE p-state ramp needs ~6us of continuous work to reach full
            # clock. Burn junk matmuls (inputs uninitialized, output never
            # read) while the first input DMAs land so the real projections
            # start at 2.4GHz instead of 0.65.
            for w in range(24):
                wps = fillps.tile([P, 512], F32, tag="fill", name=f"warm{w}")
                nc.tensor.matmul(
                    wps[0:DH, :], lhsT=ones_t[:, :],
                    rhs=yTp_t[:, 0, 0:512], start=True, stop=True,
                )
            # ---- chunk-0 projections (no attention to interleave with) ------------
            for f in proj_chunk_steps(0):
                f()

            # ---- main loop: attention(qc) with filler -----------------------------
            for qc in range(QC):
                nkt = 4 * qc + 4
                qs_lo = qc * 512

                # filler for this qc: proj of chunk qc+1 (the last 2 v units
                # of chunk 3 are held back to fill qc=3's early steps), plus
                # deferred norm(qc-1) + outproj(qc-1)
                if qc + 1 < QC:
                    vr = (0, 2) if qc + 1 == 3 else (0, 4)
                    proj_fill = proj_chunk_steps(qc + 1, v_range=vr)
                elif qc == QC - 1:
                    proj_fill = proj_chunk_steps(qc, qk=False, v_range=(2, 4))
                else:
                    proj_fill = []
                late_fill = []
                reserve = [[], []]
                if qc > 0:
                    op_prev = outproj_steps(qc - 1)
                    if qc == QC - 1:
                        # hold back 6 outproj units as ready PE filler around
                        # the inline qc=3 normalization; the rest of the late
                        # filler must be FULLY emitted before the reserve is
                        # consumed (PE is in-order: emitting a dependent
                        # instruction before its upstream PE work deadlocks)
                        late_fill = norm_steps(qc - 1, rec_alls[qc - 1]) \
                            + op_prev[:6]
                        reserve = [op_prev[6:15], op_prev[15:24]]
                        # (pair1's reserve also covers the post-attention
                        # chain; pair0's overlaps hp1 scores anyway)
                    else:
                        late_fill = norm_steps(qc - 1, rec_alls[qc - 1]) \
                            + op_prev

                n_steps = 2 * nkt
                # late filler starts once the rec DMA chain of qc-1 has had
                # time to land (the chain is ~10us; qc>=2 has longer rounds
                # so push the start out further to avoid stalling on it)
                late_start = 6 if qc < 2 else 9

                fill_i = [0, 0]

                # qc=3's held-back v units must land before their AV step
                # (~step 14), so front-load them into the first 8 steps
                proj_subs = 16 if qc == QC - 1 else 2 * n_steps
                # for qc=3, finish the late filler by sub 24 so the reserved
                # outproj units (consumed at pair boundaries) are emitted
                # strictly after their norm(2) dependencies
                late_end = 24 if qc == QC - 1 else 2 * n_steps

                def emit_filler(sub_idx, n_subs=2 * n_steps,
                                proj_fill=proj_fill, late_fill=late_fill,
                                late_sub=2 * late_start, fill_i=fill_i,
                                proj_subs=proj_subs, late_end=late_end):
                    # proj filler: spread evenly over the first proj_subs
                    want = ((sub_idx + 1) * len(proj_fill)) // proj_subs
                    want = min(want, len(proj_fill))
                    while fill_i[0] < want:
                        proj_fill[fill_i[0]]()
                        fill_i[0] += 1
                    # late filler: spread over [late_sub, late_end)
                    if sub_idx >= late_sub and late_end > late_sub:
                        span = late_end - late_sub
                        want = ((min(sub_idx, late_end - 1) - late_sub + 1)
                                * len(late_fill)) // span
                        want = min(want, len(late_fill))
                        while fill_i[1] < want:
                            late_fill[fill_i[1]]()
                            fill_i[1] += 1

                step_idx = 0
                for hp in range(2):
                    hA, hB = 2 * hp, 2 * hp + 1
                    yA = yps.tile([P, 512], F32, tag="y", name=f"yA_{qc}_{hp}")
                    yB = yps.tile([P, 512], F32, tag="y", name=f"yB_{qc}_{hp}")
                    pts = {}
                    for kc in range(nkt):
                        d = kc - 4 * qc          # straddle index (valid if >= 0)
                        lo = max(0, 128 * d)
                        for h, hb in ((hA, 0), (hB, DH)):
                            st = stps.tile([P, 512], F32, tag="st",
                                           name=f"st_{qc}_{h}_{kc}")
                            nc.tensor.matmul(
                                st[:, lo:],
                                lhsT=kT_t[hb:hb + DH, hp, kc * P:(kc + 1) * P],
                                rhs=qT_t[hb:hb + DH, hp, qs_lo + lo:qs_lo + 512],
                                start=True, stop=True,
                            )
                            pt = ptpool.tile([P, 512], F16, tag="pt",
                                             name=f"pt_{qc}_{h}_{kc}")
                            nc.scalar.activation(
                                pt[:, lo:], st[:, lo:],
                                mybir.ActivationFunctionType.Exp,
                                scale=float(SCALE),
                            )
                            if d >= 0:
                                if lo > 0:
                                    nc.gpsimd.memset(pt[:, :lo], 0.0)
                                nc.gpsimd.tensor_mul(
                                    out=pt[:, lo:lo + P],
                                    in0=pt[:, lo:lo + P],
                                    in1=mask_t[:, :],
                                )
                            pts[(h, kc)] = pt
                        emit_filler(2 * step_idx)
                        if kc > 0:
                            for h, y in ((hA, yA), (hB, yB)):
                                nc.tensor.matmul(
                                    y[0:DH + 1, :],
                                    lhsT=v_t[:, kc - 1, h, :],
                                    rhs=pts.pop((h, kc - 1))[:, :],
                                    start=(kc - 1 == 0),
                                    stop=False,
                                )
                        emit_filler(2 * step_idx + 1)
                        step_idx += 1
                    # tail AV for kc = nkt-1
                    for h, y in ((hA, yA), (hB, yB)):
                        nc.tensor.matmul(
                            y[0:DH + 1, :],
                            lhsT=v_t[:, nkt - 1, h, :],
                            rhs=pts.pop((h, nkt - 1))[:, :],
                            start=(nkt == 1),
                            stop=True,
                        )
                    # drain y to SBUF (rows 0:65 even / 63:128 odd), free psum
                    yuA = yupool.tile([DH + 1, 512], F32, tag="yu",
                                      name=f"yu_{qc}_{hA}")
                    yuB = yupool.tile([DH + 1, 512], F32, tag="yu",
                                      name=f"yu_{qc}_{hB}")
                    nc.vector.tensor_copy(out=yuA[0:DH + 1, :], in_=yA[0:DH + 1, :])
                    nc.vector.tensor_copy(out=yuB[0:DH + 1, :], in_=yB[0:DH + 1, :])
                    yu_tiles[qc][hA] = yuA
                    yu_tiles[qc][hB] = yuB
                    if qc == QC - 1:
                        inline_norm_pair(qc, hp, yuA, yuB, (yA, yB),
                                         reserve[hp])

                if qc == QC - 1:
                    continue
                # reciprocal of the 4 sums rows via DRAM-bounce relayout
                sums_dram = drampool.tile([HPC, 512], F32, tag="sums_dram")
                for h in range(HPC):
                    yu = yu_tiles[qc][h]
                    nc.sync.dma_start(
                        out=sums_dram[h:h + 1, :], in_=yu[DH:DH + 1, :]
                    )
                s_resh = recpool.tile([P, HPC, 4], F32, tag="sresh")
                nc.sync.dma_start(
                    out=s_resh[:],
                    in_=sums_dram.rearrange("h (p j) -> p h j", p=P),
                )
                r_resh = recpool.tile([P, HPC, 4], F16, tag="rresh")
                nc.vector.reciprocal(r_resh[:], s_resh[:])
                rec_dram = drampool.tile([HPC, 512], F16, tag="rec_dram")
                nc.sync.dma_start(
                    out=rec_dram.rearrange("h (p j) -> p h j", p=P),
                    in_=r_resh[:],
                )
                rec_all = recpool.tile([1, HPC, 512], F16, tag="recall")
                nc.sync.dma_start(out=rec_all[0:1, :, :], in_=rec_dram[None, :, :])
                rec_alls[qc] = rec_all

            # ---- tail: outproj for the last q-chunk -------------------------------
            for f in outproj_steps(QC - 1):
                f()

    return nc


def _make_mask():
    kp = np.arange(P)[:, None]
    i = np.arange(P)[None, :]
    return (kp <= i).astype(np.float16)


def kernel(x, pos_emb, Wq, Wk, Wv, Wo):
    global _NC_CACHE, LAST_RESULT
    x = np.asarray(x, dtype=np.float32)
    pos_emb = np.asarray(pos_emb, dtype=np.float32)
    Wq = np.asarray(Wq, dtype=np.float32)
    Wk = np.asarray(Wk, dtype=np.float32)
    Wv = np.asarray(Wv, dtype=np.float32)
    Wo = np.asarray(Wo, dtype=np.float32)

    if _NC_CACHE is None:
        _NC_CACHE = _build_nc()
    nc = _NC_CACHE

    mask = _make_mask()
    xT = [np.ascontiguousarray(x[b].T).astype(np.float16) for b in range(B)]
    in_maps = []
    for c in range(N_CORES):
        b, g = divmod(c, 4)
        ch = slice(g * CS, (g + 1) * CS)
        in_maps.append({
            "xT": xT[b],
            "wqT": np.ascontiguousarray(Wq[ch, :].T).astype(np.float16),
            "wkT": np.ascontiguousarray(Wk[ch, :].T).astype(np.float16),
            "wvT": np.ascontiguousarray(Wv[ch, :].T).astype(np.float16),
            "peT": np.ascontiguousarray(pos_emb[:T, ch].T).astype(np.float16),
            "pen": np.ascontiguousarray(pos_emb[:T, ch]).astype(np.float16),
            "woT": np.ascontiguousarray(Wo[:, ch].T).astype(np.float16),
            "maskT": mask,
        })

    res = run_bass_kernel_spmd(
        nc, in_maps, list(range(N_CORES)), trace=TRACE, **TRACE_KWARGS
    )
    LAST_RESULT = res

    out = np.zeros((B, T, C), dtype=np.float32)
    for c in range(N_CORES):
        b = c // 4
        out[b] += res.results[c]["out"].astype(np.float32)
    return out


# revision 21
# speedup vs baseline: 1.0358x; 1.0358x over previous
"""Causal self-attention (B=2, T=2048, C=1024, 16 heads) on 8 trn2 NeuronCores.

Sharding: tensor-parallel, core c = b*4+g handles batch b (2) x head-group g
(4 heads = 256 channels). Each core computes q/k/v projections for its
channels, causal attention for its 4 heads, and the slice of the output
projection contracting its channels. Host sums the 4 partial outputs per
batch. No cross-core communication on device.

Single-core schedule is built around keeping the PE array continuously fed
(the TRN2 PE DVFS ramp only reaches full clock after ~3us of uninterrupted
work): attention score/AV matmuls for a head pair are pipelined with lag-1
against the ACT-engine exps, and projection / output-projection matmuls are
dribbled between attention steps as filler. All matmul operands are fp16.
"""

import sys

if "/opt/trn_rl_repo" not in sys.path:
    sys.path.insert(0, "/opt/trn_rl_repo")

import numpy as np

import concourse.bass as bass
import concourse.mybir as mybir
from concourse.bass_utils import run_bass_kernel_spmd
from concourse.tile import TileContext
import concourse.tile_utils as _tile_utils

_tile_utils.max_sbuf_usage = 208 * 1024
from concourse.vector_clock import ScopedClock

# ---------------------------------------------------------------------------
# Walrus on this image rejects >4 sem waits on a single instruction; the stock
# TileContext tail-drain attaches one wait per active logical processor.
# Split them into standalone wait_ge instructions instead.
def _patched_drain_and_barrier(self, tick_clock, wait_clock):
    probe = mybir.InstNoOp(name="wait_probe", ins=[], outs=[])
    probe.engine = mybir.EngineType.SP
    wait_clock.add_sem_waits(probe, ScopedClock({None: tick_clock.global_clock}))
    waits = (
        list(probe.sync_info.on_wait)
        if probe.sync_info and probe.sync_info.on_wait
        else []
    )
    assert self.sems is not None
    sem_by_num = {s.num: s for s in self.sems.allocated().values()}
    for w in waits:
        assert w.wait_mode == "sem-ge-imm", w
        self.nc.sync.wait_ge(sem_by_num[w.id], w.wait_value)
    self.nc.sync.drain()
    self.nc.all_engine_barrier()
    popped = self.nc._tile_sem_poison_stack.pop()
    assert popped is self._sem_poison
    self.nc.clear_and_free_semaphores(list(self.sems.allocated().values()))
    self.nc.all_engine_barrier()


TileContext._drain_and_barrier = _patched_drain_and_barrier

# The same walrus limit applies to regular instructions (matmul/LDWEIGHTS
# rejects even 2 waits). Split multi-wait instructions: excess waits move to
# single-wait NoOps committed just before on the same engine.
_orig_commit = TileContext._commit_instruction


def _split_commit(self, inst, lazy_reg_writes=True):
    si = inst.sync_info
    if (
        si is not None
        and si.on_wait
        and len(si.on_wait) > 1
        and inst.engine != mybir.EngineType.Unassigned
    ):
        waits = list(si.on_wait)
        for w in waits[:-1]:
            nop = mybir.InstNoOp(
                name=self.nc.get_next_instruction_name(),
                ins=[],
                outs=[],
                engine=inst.engine,
                sync_info=mybir.SyncInfo(on_wait=[w], on_update=[]),
                bass_nofuse=True,
            )
            _orig_commit(self, nop, lazy_reg_writes=False)
        inst.sync_info = mybir.SyncInfo(
            on_wait=[waits[-1]], on_update=list(si.on_update or [])
        )
    _orig_commit(self, inst, lazy_reg_writes)


TileContext._commit_instruction = _split_commit
# ---------------------------------------------------------------------------

N_CORES = 8
B, T, C = 2, 2048, 1024
H = 16
DH = C // H                       # 64
HPC = H // 4                      # 4 heads per core
CS = HPC * DH                     # 256 channels per core
SCALE = 1.0 / np.sqrt(np.float32(C))  # note: sqrt(n_embd), per reference

P = 128                           # partitions
TB = T // P                       # 16 t-blocks of 128
QC = T // 512                     # 4 q-chunks of 512
KO = C // P                       # 8 contraction subtiles for projections

F32 = mybir.dt.float32
F16 = mybir.dt.float16

TRACE = False        # test.py flips this to profile
TRACE_KWARGS = {}
LAST_RESULT = None   # BassKernelResults of the most recent run

_NC_CACHE = None


def _build_nc():
    nc = bass.Bass()

    xT_d = nc.dram_tensor("xT", [C, T], F16, kind="ExternalInput")
    wqT_d = nc.dram_tensor("wqT", [C, CS], F16, kind="ExternalInput")
    wkT_d = nc.dram_tensor("wkT", [C, CS], F16, kind="ExternalInput")
    wvT_d = nc.dram_tensor("wvT", [C, CS], F16, kind="ExternalInput")
    peT_d = nc.dram_tensor("peT", [CS, T], F16, kind="ExternalInput")
    pen_d = nc.dram_tensor("pen", [T, CS], F16, kind="ExternalInput")
    woT_d = nc.dram_tensor("woT", [CS, C], F16, kind="ExternalInput")
    mask_d = nc.dram_tensor("maskT", [P, P], F16, kind="ExternalInput")
    out_d = nc.dram_tensor("out", [T, C], F16, kind="ExternalOutput")

    with TileContext(nc) as tc:
        with (
            nc.allow_low_precision(reason="fp16 matmul pipeline by design"),
            tc.tile_pool(name="const", bufs=1) as const,
            tc.tile_pool(name="ptp", bufs=6) as ptpool,
            tc.tile_pool(name="yup", bufs=6) as yupool,
            tc.tile_pool(name="recp", bufs=2) as recpool,
            tc.tile_pool(name="oddp", bufs=2) as oddpool,
            tc.tile_pool(name="outp", bufs=3) as outpool,
            tc.tile_pool(name="stps", bufs=3, space="PSUM") as stps,
            tc.tile_pool(name="yps", bufs=2, space="PSUM") as yps,
            tc.tile_pool(name="fillps", bufs=3, space="PSUM") as fillps,
            tc.tile_pool(name="scratch", bufs=2, space="DRAM") as drampool,
        ):
            # ---- persistent tiles -------------------------------------------------
            wq_t = const.tile([P, KO, CS], F16, tag="wq")
            wk_t = const.tile([P, KO, CS], F16, tag="wk")
            wv_t = const.tile([P, KO, CS], F16, tag="wv")
            wo_t = const.tile([P, 2, C], F16, tag="wo")
            peT_t = const.tile([P, 2, T], F16, tag="peT")
            pen_t = const.tile([P, TB, CS], F16, tag="pen")
            mask_t = const.tile([P, P], F16, tag="mask")
            qT_t = const.tile([P, 2, T], F16, tag="qT")
            kT_t = const.tile([P, 2, T], F16, tag="kT")
            # per-head v with a trailing ones column ([v|1]) for row sums
            v_t = const.tile([P, TB, HPC, DH + 1], F16, tag="v")
            yTp_t = const.tile([P, 2, T], F16, tag="yTp")
            ones_t = const.tile([P, DH], F16, tag="ones")

            x_ts = [const.tile([P, KO, 512], F16, tag=f"x{n}", name=f"x_{n}")
                    for n in range(QC)]

            xT_r = xT_d.rearrange("(o p) t -> p o t", p=P)

            # DMA order = need order; first x chunk + wq halves split so the
            # first projection matmuls can start ASAP; x0 goes through the ACT
            # engine's DGE so its config isn't serialized behind the weights'
            # on the SP sequencer.
            wq_r = wqT_d.rearrange("(o p) m -> p o m", p=P)
            nc.scalar.dma_start(out=x_ts[0][:, 0:4, :], in_=xT_r[:, 0:4, 0:512])
            nc.sync.dma_start(out=wq_t[:, :, 0:P], in_=wq_r[:, :, 0:P])
            nc.gpsimd.dma_start(out=peT_t[:], in_=peT_d.rearrange("(o p) m -> p o m", p=P))
            nc.scalar.dma_start(out=x_ts[0][:, 4:8, :], in_=xT_r[:, 4:8, 0:512])
            nc.sync.dma_start(out=wq_t[:, :, P:CS], in_=wq_r[:, :, P:CS])
            nc.sync.dma_start(out=wk_t[:], in_=wkT_d.rearrange("(o p) m -> p o m", p=P))
            nc.gpsimd.dma_start(out=pen_t[:], in_=pen_d.rearrange("(o p) m -> p o m", p=P))
            nc.sync.dma_start(out=wv_t[:], in_=wvT_d.rearrange("(o p) m -> p o m", p=P))
            for n in range(1, QC):
                nc.sync.dma_start(out=x_ts[n][:], in_=xT_r[:, :, n * 512:(n + 1) * 512])
            nc.sync.dma_start(out=mask_t[:], in_=mask_d[:, :])
            nc.sync.dma_start(out=wo_t[:], in_=woT_d.rearrange("(o p) m -> p o m", p=P))

            nc.gpsimd.memset(ones_t[:], 1.0)
            nc.gpsimd.memset(v_t[:, :, :, DH:DH + 1], 1.0)
            # warm the combined ln/exp ACT table during the input-DMA wait so
            # the first real exp doesn't pay the 1.3us table load
            warm_t = const.tile([1, 2], F32, tag="warm")
            nc.scalar.activation(warm_t[0:1, 0:1], ones_t[0:1, 0:1],
                                 mybir.ActivationFunctionType.Exp)
            nc.scalar.activation(warm_t[0:1, 1:2], ones_t[0:1, 0:1],
                                 mybir.ActivationFunctionType.Ln)

            # ---- filler streams ---------------------------------------------------
            # Each filler step is a zero-arg callable emitting ONE instruction
            # (matmul / vector drain / dma). Attention steps consume these to
            # keep the PE queue backlogged with independent work.

            def proj_chunk_steps(n, qk=True, v_range=(0, 4)):
                """q/k/v projections for t-chunk n: list of step callables.

                qk=False skips the q/k units; v_range selects which v units
                to emit (v units can be deferred into the owning q-chunk's
                attention steps since AV needs v[tb] only at step tb).
                """
                steps = []
                ts = slice(n * 512, (n + 1) * 512)
                x_t = x_ts[n]

                qk_pairs = [(wq_t, qT_t), (wk_t, kT_t)] if qk else []
                for (w_t, dst) in qk_pairs:
                    for m in range(2):
                        box = {}

                        def mk_mm(ko, w_t=w_t, m=m, box=box):
                            def f():
                                if ko == 0:
                                    box["ps"] = fillps.tile(
                                        [P, 512], F32, tag="fill", name="psqk")
                                nc.tensor.matmul(
                                    box["ps"][:],
                                    lhsT=w_t[:, ko, m * P:(m + 1) * P],
                                    rhs=x_t[:, ko, :],
                                    start=(ko == 0),
                                    stop=(ko == KO - 1),
                                )
                            return f

                        for ko in range(KO):
                            steps.append(mk_mm(ko))

                        def mk_drain(dst=dst, m=m, ts=ts, box=box):
                            def f():
                                nc.vector.tensor_add(
                                    out=dst[:, m, ts], in0=box["ps"][:],
                                    in1=peT_t[:, m, ts],
                                )
                            return f

                        steps.append(mk_drain())

                for tb4 in range(*v_range):
                    tb = n * 4 + tb4
                    box = {}

                    def mk_mmv(ko, tb4=tb4, box=box):
                        def f():
                            if ko == 0:
                                box["ps"] = fillps.tile(
                                    [P, 512], F32, tag="fill", name="psv")
                            nc.tensor.matmul(
                                box["ps"][:, :CS],
                                lhsT=x_t[:, ko, tb4 * P:(tb4 + 1) * P],
                                rhs=wv_t[:, ko, :],
                                start=(ko == 0),
                                stop=(ko == KO - 1),
                            )
                        return f

                    for ko in range(KO):
                        steps.append(mk_mmv(ko))

                    def mk_drainv(tb=tb, box=box):
                        def f():
                            nc.vector.tensor_add(
                                out=v_t[:, tb, :, 0:DH],
                                in0=box["ps"][:, :CS].rearrange(
                                    "p (a b) -> p a b", a=HPC),
                                in1=pen_t[:, tb, :].rearrange(
                                    "p (a b) -> p a b", a=HPC),
                            )
                        return f

                    steps.append(mk_drainv())
                return steps

            def norm_steps(qc, rec_all):
                """Deferred normalization for q-chunk qc (bcast + muls).
                Odd heads are normalized into a staging tile then partition-
                shifted to rows 64-127 of yTp via SBUF->SBUF DMA."""
                qs = slice(qc * 512, (qc + 1) * 512)
                steps = []
                for hp in range(2):
                    box = {}

                    def mk_bc(hp=hp, box=box):
                        def f():
                            box["bcA"] = fillps.tile([P, 512], F32, tag="fill",
                                                     name="bcA")
                            box["bcB"] = fillps.tile([P, 512], F32, tag="fill",
                                                     name="bcB")
                            nc.tensor.matmul(
                                box["bcA"][0:DH, :], lhsT=ones_t[0:1, :],
                                rhs=rec_all[0:1, 2 * hp, :],
                                start=True, stop=True,
                            )
                            nc.tensor.matmul(
                                box["bcB"][0:DH, :], lhsT=ones_t[0:1, :],
                                rhs=rec_all[0:1, 2 * hp + 1, :],
                                start=True, stop=True,
                            )
                        return f

                    def mk_mul(hp=hp, box=box):
                        yuA, yuB = yu_tiles[qc][2 * hp], yu_tiles[qc][2 * hp + 1]

                        def f():
                            nc.vector.tensor_mul(
                                out=yTp_t[0:DH, hp, qs],
                                in0=yuA[0:DH, :], in1=box["bcA"][0:DH, :],
                            )
                            ytn = oddpool.tile([DH, 512], F16, tag="ytn",
                                               name=f"ytn_{qc}_{hp}")
                            nc.vector.tensor_mul(
                                out=ytn[:], in0=yuB[0:DH, :],
                                in1=box["bcB"][0:DH, :],
                            )
                            nc.sync.dma_start(
                                out=yTp_t[DH:P, hp, qs], in_=ytn[:])
                        return f

                    steps.append(mk_bc())
                    steps.append(mk_mul())
                return steps

            def outproj_steps(qc):
                """Output projection for q-chunk qc: 8 units of (2mm+drain)."""
                steps = []
                for tb4 in range(4):
                    tb = qc * 4 + tb4
                    tsl = slice(tb * P, (tb + 1) * P)
                    for oc in range(2):
                        box = {}
                        ocs = slice(oc * 512, (oc + 1) * 512)

                        def mk_mm(m, tsl=tsl, ocs=ocs, box=box):
                            def f():
                                if m == 0:
                                    box["ps"] = fillps.tile(
                                        [P, 512], F32, tag="fill", name="pso")
                                nc.tensor.matmul(
                                    box["ps"][:],
                                    lhsT=yTp_t[:, m, tsl],
                                    rhs=wo_t[:, m, ocs],
                                    start=(m == 0),
                                    stop=(m == 1),
                                )
                            return f

                        def mk_drain(tb=tb, oc=oc, tsl=tsl, ocs=ocs, box=box):
                            def f():
                                o_t = outpool.tile([P, 512], F16, tag="out",
                                                   name=f"o_{tb}_{oc}")
                                nc.vector.tensor_copy(out=o_t[:], in_=box["ps"][:])
                                nc.sync.dma_start(out=out_d[tsl, ocs], in_=o_t[:])
                            return f

                        steps.append(mk_mm(0))
                        steps.append(mk_mm(1))
                        steps.append(mk_drain())
                return steps


            def inline_norm_pair(qc, hp, yuA, yuB, yps_pair, reserve):
                """qc=3 normalization for pair hp, engine-local (no DMA
                bounce): reciprocal_approx_fast on the sums lane (partition
                64), fp16 copy, ones-bcast matmul at base partition 64.
                `reserve` = ready PE filler steps interleaved to cover the
                cross-engine latency."""
                qs = slice(qc * 512, (qc + 1) * 512)
                ri = iter(reserve)

                def take(k):
                    for _ in range(k):
                        f = next(ri, None)
                        if f is not None:
                            f()

                # 1/s = exp(-ln(s)) on the ACT engine: stays on the sums
                # lane (partition 64), no relayout, and the
                # natural_log_exp_and_others act table covers both functions
                rr, rf = {}, {}
                for tag, ysrc in (("A", yps_pair[0]), ("B", yps_pair[1])):
                    rr[tag] = recpool.tile([DH + 1, 512], F32, tag="rr",
                                           name=f"rr{tag}_{hp}")
                    nc.scalar.activation(
                        rr[tag][DH:DH + 1, :], ysrc[DH:DH + 1, :],
                        mybir.ActivationFunctionType.Ln,
                    )
                take(3)
                for tag in ("A", "B"):
                    rf[tag] = recpool.tile([DH + 1, 512], F16, tag="rf",
                                           name=f"rf{tag}_{hp}")
                    nc.scalar.activation(
                        rf[tag][DH:DH + 1, :], rr[tag][DH:DH + 1, :],
                        mybir.ActivationFunctionType.Exp, scale=-1.0,
                    )
                take(3)
                bcA = fillps.tile([P, 512], F32, tag="fill", name="bcA3")
                bcB = fillps.tile([P, 512], F32, tag="fill", name="bcB3")
                nc.tensor.matmul(bcA[0:DH, :], lhsT=ones_t[DH:DH + 1, :],
                                 rhs=rf["A"][DH:DH + 1, :],
                                 start=True, stop=True)
                nc.tensor.matmul(bcB[0:DH, :], lhsT=ones_t[DH:DH + 1, :],
                                 rhs=rf["B"][DH:DH + 1, :],
                                 start=True, stop=True)
                nc.vector.tensor_mul(
                    out=yTp_t[0:DH, hp, qs], in0=yuA[0:DH, :], in1=bcA[0:DH, :])
                ytn = oddpool.tile([DH, 512], F16, tag="ytn",
                                   name=f"ytn3_{hp}")
                nc.vector.tensor_mul(out=ytn[:], in0=yuB[0:DH, :],
                                     in1=bcB[0:DH, :])
                nc.sync.dma_start(out=yTp_t[DH:P, hp, qs], in_=ytn[:])
                take(9)

            yu_tiles = [[None] * HPC for _ in range(QC)]
            rec_alls = [None] * QC

            # ---- PE warm-up -------------------------------------------------------
            # The PE p-state ramp needs ~6us of continuous work to reach full
            # clock. Burn junk matmuls (inputs uninitialized, output never
            # read) while the first input DMAs land so the real projections
            # start at 2.4GHz instead of 0.65.
            for w in range(14):
                wps = fillps.tile([P, 512], F32, tag="fill", name=f"warm{w}")
                nc.tensor.matmul(
                    wps[0:DH, :], lhsT=ones_t[:, :],
                    rhs=yTp_t[:, 0, 0:512], start=True, stop=True,
                )
            # ---- chunk-0 projections (no attention to interleave with) ------------
            for f in proj_chunk_steps(0):
                f()

            # ---- main loop: attention(qc) with filler -----------------------------
            for qc in range(QC):
                nkt = 4 * qc + 4
                qs_lo = qc * 512

                # filler for this qc: proj of chunk qc+1 (the last 2 v units
                # of chunk 3 are held back to fill qc=3's early steps), plus
                # deferred norm(qc-1) + outproj(qc-1)
                if qc + 1 < QC:
                    vr = (0, 2) if qc + 1 == 3 else (0, 4)
                    proj_fill = proj_chunk_steps(qc + 1, v_range=vr)
                elif qc == QC - 1:
                    proj_fill = proj_chunk_steps(qc, qk=False, v_range=(2, 4))
                else:
                    proj_fill = []
                late_fill = []
                reserve = [[], []]
                if qc > 0:
                    op_prev = outproj_steps(qc - 1)
                    if qc == QC - 1:
                        # hold back 6 outproj units as ready PE filler around
                        # the inline qc=3 normalization; the rest of the late
                        # filler must be FULLY emitted before the reserve is
                        # consumed (PE is in-order: emitting a dependent
                        # instruction before its upstream PE work deadlocks)
                        late_fill = norm_steps(qc - 1, rec_alls[qc - 1]) \
                            + op_prev[:6]
                        reserve = [op_prev[6:15], op_prev[15:24]]
                        # (pair1's reserve also covers the post-attention
                        # chain; pair0's overlaps hp1 scores anyway)
                    else:
                        late_fill = norm_steps(qc - 1, rec_alls[qc - 1]) \
                            + op_prev

                n_steps = 2 * nkt
                # late filler starts once the rec DMA chain of qc-1 has had
                # time to land (the chain is ~10us; qc>=2 has longer rounds
                # so push the start out further to avoid stalling on it)
                late_start = 8 if qc < 2 else 9

                fill_i = [0, 0]

                # qc=3's held-back v units must land before their AV step
                # (~step 14), so front-load them into the first 8 steps
                proj_subs = 16 if qc == QC - 1 else 2 * n_steps
                # for qc=3, finish the late filler by sub 24 so the reserved
                # outproj units (consumed at pair boundaries) are emitted
                # strictly after their norm(2) dependencies
                late_end = 24 if qc == QC - 1 else 2 * n_steps

                def emit_filler(sub_idx, n_subs=2 * n_steps,
                                proj_fill=proj_fill, late_fill=late_fill,
                                late_sub=2 * late_start, fill_i=fill_i,
                                proj_subs=proj_subs, late_end=late_end):
                    # proj filler: spread evenly over the first proj_subs
                    want = ((sub_idx + 1) * len(proj_fill)) // proj_subs
                    want = min(want, len(proj_fill))
                    while fill_i[0] < want:
                        proj_fill[fill_i[0]]()
                        fill_i[0] += 1
                    # late filler: spread over [late_sub, late_end)
                    if sub_idx >= late_sub and late_end > late_sub:
                        span = late_end - late_sub
                        want = ((min(sub_idx, late_end - 1) - late_sub + 1)
                                * len(late_fill)) // span
                        want = min(want, len(late_fill))
                        while fill_i[1] < want:
                            late_fill[fill_i[1]]()
                            fill_i[1] += 1

                step_idx = 0
                for hp in range(2):
                    hA, hB = 2 * hp, 2 * hp + 1
                    yA = yps.tile([P, 512], F32, tag="y", name=f"yA_{qc}_{hp}")
                    yB = yps.tile([P, 512], F32, tag="y", name=f"yB_{qc}_{hp}")
                    pts = {}
                    for kc in range(nkt):
                        d = kc - 4 * qc          # straddle index (valid if >= 0)
                        lo = max(0, 128 * d)
                        for h, hb in ((hA, 0), (hB, DH)):
                            st = stps.tile([P, 512], F32, tag="st",
                                           name=f"st_{qc}_{h}_{kc}")
                            nc.tensor.matmul(
                                st[:, lo:],
                                lhsT=kT_t[hb:hb + DH, hp, kc * P:(kc + 1) * P],
                                rhs=qT_t[hb:hb + DH, hp, qs_lo + lo:qs_lo + 512],
                                start=True, stop=True,
                            )
                            pt = ptpool.tile([P, 512], F16, tag="pt",
                                             name=f"pt_{qc}_{h}_{kc}")
                            nc.scalar.activation(
                                pt[:, lo:], st[:, lo:],
                                mybir.ActivationFunctionType.Exp,
                                scale=float(SCALE),
                            )
                            if d >= 0:
                                if lo > 0:
                                    nc.gpsimd.memset(pt[:, :lo], 0.0)
                                nc.gpsimd.tensor_mul(
                                    out=pt[:, lo:lo + P],
                                    in0=pt[:, lo:lo + P],
                                    in1=mask_t[:, :],
                                )
                            pts[(h, kc)] = pt
                        emit_filler(2 * step_idx)
                        if kc > 0:
                            for h, y in ((hA, yA), (hB, yB)):
                                nc.tensor.matmul(
                                    y[0:DH + 1, :],
                                    lhsT=v_t[:, kc - 1, h, :],
                                    rhs=pts.pop((h, kc - 1))[:, :],
                                    start=(kc - 1 == 0),
                                    stop=False,
                                )
                        emit_filler(2 * step_idx + 1)
                        step_idx += 1
                    # tail AV for kc = nkt-1
                    for h, y in ((hA, yA), (hB, yB)):
                        nc.tensor.matmul(
                            y[0:DH + 1, :],
                            lhsT=v_t[:, nkt - 1, h, :],
                            rhs=pts.pop((h, nkt - 1))[:, :],
                            start=(nkt == 1),
                            stop=True,
                        )
                    # drain y to SBUF (rows 0:65 even / 63:128 odd), free psum
                    yuA = yupool.tile([DH + 1, 512], F32, tag="yu",
                                      name=f"yu_{qc}_{hA}")
                    yuB = yupool.tile([DH + 1, 512], F32, tag="yu",
                                      name=f"yu_{qc}_{hB}")
                    nc.vector.tensor_copy(out=yuA[0:DH + 1, :], in_=yA[0:DH + 1, :])
                    nc.vector.tensor_copy(out=yuB[0:DH + 1, :], in_=yB[0:DH + 1, :])
                    yu_tiles[qc][hA] = yuA
                    yu_tiles[qc][hB] = yuB
                    if qc == QC - 1:
                        inline_norm_pair(qc, hp, yuA, yuB, (yA, yB),
                                         reserve[hp])

                if qc == QC - 1:
                    continue
                # reciprocal of the 4 sums rows via DRAM-bounce relayout
                sums_dram = drampool.tile([HPC, 512], F32, tag="sums_dram")
                for h in range(HPC):
                    yu = yu_tiles[qc][h]
                    nc.sync.dma_start(
                        out=sums_dram[h:h + 1, :], in_=yu[DH:DH + 1, :]
                    )
                s_resh = recpool.tile([P, HPC, 4], F32, tag="sresh")
                nc.sync.dma_start(
                    out=s_resh[:],
                    in_=sums_dram.rearrange("h (p j) -> p h j", p=P),
                )
                r_resh = recpool.tile([P, HPC, 4], F16, tag="rresh")
                nc.vector.reciprocal(r_resh[:], s_resh[:])
                rec_dram = drampool.tile([HPC, 512], F16, tag="rec_dram")
                nc.sync.dma_start(
                    out=rec_dram.rearrange("h (p j) -> p h j", p=P),
                    in_=r_resh[:],
                )
                rec_all = recpool.tile([1, HPC, 512], F16, tag="recall")
                nc.sync.dma_start(out=rec_all[0:1, :, :], in_=rec_dram[None, :, :])
                rec_alls[qc] = rec_all

            # ---- tail: outproj for the last q-chunk -------------------------------
            for f in outproj_steps(QC - 1):
                f()

    return nc


def _make_mask():
    kp = np.arange(P)[:, None]
    i = np.arange(P)[None, :]
    return (kp <= i).astype(np.float16)


def kernel(x, pos_emb, Wq, Wk, Wv, Wo):
    global _NC_CACHE, LAST_RESULT
    x = np.asarray(x, dtype=np.float32)
    pos_emb = np.asarray(pos_emb, dtype=np.float32)
    Wq = np.asarray(Wq, dtype=np.float32)
    Wk = np.asarray(Wk, dtype=np.float32)
    Wv = np.asarray(Wv, dtype=np.float32)
    Wo = np.asarray(Wo, dtype=np.float32)

    if _NC_CACHE is None:
        _NC_CACHE = _build_nc()
    nc = _NC_CACHE

    mask = _make_mask()
    xT = [np.ascontiguousarray(x[b].T).astype(np.float16) for b in range(B)]
    in_maps = []
    for c in range(N_CORES):
        b, g = divmod(c, 4)
        ch = slice(g * CS, (g + 1) * CS)
        in_maps.append({
            "xT": xT[b],
            "wqT": np.ascontiguousarray(Wq[ch, :].T).astype(np.float16),
            "wkT": np.ascontiguousarray(Wk[ch, :].T).astype(np.float16),
            "wvT": np.ascontiguousarray(Wv[ch, :].T).astype(np.float16),
            "peT": np.ascontiguousarray(pos_emb[:T, ch].T).astype(np.float16),
            "pen": np.ascontiguousarray(pos_emb[:T, ch]).astype(np.float16),
            "woT": np.ascontiguousarray(Wo[:, ch].T).astype(np.float16),
            "maskT": mask,
        })

    res = run_bass_kernel_spmd(
        nc, in_maps, list(range(N_CORES)), trace=TRACE, **TRACE_KWARGS
    )
    LAST_RESULT = res

    out = np.zeros((B, T, C), dtype=np.float32)
    for c in range(N_CORES):
        b = c // 4
        out[b] += res.results[c]["out"].astype(np.float32)
    return out
